# revision 69
# baseline (speedup 1.0000x reference)
"""Trainium2 Bass kernel for CrossViewAttention (gnn message passing).

Strategy:
  - Algebra: scores[e] = Q2[qi].kv[kj] with Q2 = q @ (scale Wq^T Wk) + scale bq Wk
    (per-node-constant terms cancel under segment softmax). V/out projections
    commute with the weighted segment sum, so only RAW kv rows are gathered.
    out[n] = q[n] + (ctx[n]/denom[n]) @ (Wo Wv)^T + bvo ; q/bvo added on host.
  - Sharding: 50k query nodes -> 8 cores x 98 groups x 64 slots, 2D-balanced by
    (low,high) edge degree so a fixed per-group layout of 9 chunks x 128 edges
    (6 low-table | 3 high-table) holds; overflow edges -> dense correction
    table (host exp) added into the group accumulator on device (DVE).
  - Device per chunk (all bf16 operands, fp32 PSUM accumulate):
    PE-transpose gathered kv, M = kvT.T @ Q2T[:,win] (PSUM), ACT exp(M),
    one DVE scalar_tensor_tensor (iota==qcol)*expM -> masked attention
    weights W, PE scatter matmuls acc += W.T @ [kv | 1].
  - Q2T computed on host (saves the on-device projection prep loop); final
    out_proj done per 2-group pair (128-row matmuls).
"""

import numpy as np
import ml_dtypes

BF16 = ml_dtypes.bfloat16

# ---------------- static structure ----------------
N = 50000
E = 800000
D = 128
NC = 8
GROUP_NODES = 64
GROUPS_PER_CORE = 98
TOTAL_GROUPS = NC * GROUPS_PER_CORE
LOCAL_NODES = GROUPS_PER_CORE * GROUP_NODES          # 6272
R_SPLIT = 31250
LOW_CHUNKS, HIGH_CHUNKS = 5, 3
CHUNKS_PER_GROUP = LOW_CHUNKS + HIGH_CHUNKS          # 9
LOW_CAP, HIGH_CAP = LOW_CHUNKS * 128, HIGH_CHUNKS * 128
GROUPS_PER_BULK = 14
BULKS = GROUPS_PER_CORE // GROUPS_PER_BULK           # 7
CHUNKS_PER_BULK = GROUPS_PER_BULK * CHUNKS_PER_GROUP # 63
CHUNKS_PER_CORE = GROUPS_PER_CORE * CHUNKS_PER_GROUP # 882
LOW_IDX_COLS = GROUPS_PER_BULK * LOW_CAP // 16       # 336
HIGH_IDX_COLS = GROUPS_PER_BULK * HIGH_CAP // 16     # 168
IDX_COLS_PER_BULK = LOW_IDX_COLS + HIGH_IDX_COLS     # 504
N_GATHER_QUEUES = 4


# ---------------- host prep ----------------
def _balance_nodes(deg_low, deg_high):
    import heapq
    order = np.argsort(-(deg_low + deg_high), kind="stable")
    glow = np.zeros(TOTAL_GROUPS, np.int64)
    ghigh = np.zeros(TOTAL_GROUPS, np.int64)
    gcnt = np.zeros(TOTAL_GROUPS, np.int64)
    group_of = np.empty(N, np.int64)
    heap = [(0.0, g) for g in range(TOTAL_GROUPS)]
    heapq.heapify(heap)
    for n in order:
        dl, dh = deg_low[n], deg_high[n]
        while True:
            key, g = heapq.heappop(heap)
            if gcnt[g] < GROUP_NODES:
                break
        group_of[n] = g
        glow[g] += dl; ghigh[g] += dh; gcnt[g] += 1
        heapq.heappush(heap, (glow[g] / LOW_CAP + ghigh[g] / HIGH_CAP, g))
    return group_of


def host_prepare(query_nodes, key_value_nodes, edge_index,
                 Wq, bq, Wk, bk, Wv, bv, Wo, bo):
    q = np.ascontiguousarray(np.asarray(query_nodes, np.float32))
    kv = np.ascontiguousarray(np.asarray(key_value_nodes, np.float32))
    qi = np.asarray(edge_index[0], np.int64)
    kj = np.asarray(edge_index[1], np.int64)
    scale = np.float64(D) ** -0.5

    Wq64, Wk64 = np.asarray(Wq, np.float64), np.asarray(Wk, np.float64)
    Wv64, Wo64 = np.asarray(Wv, np.float64), np.asarray(Wo, np.float64)
    WQK = (scale * (Wq64.T @ Wk64)).astype(np.float32)
    vq = (scale * (np.asarray(bq, np.float64) @ Wk64)).astype(np.float32)
    WvoT = np.ascontiguousarray((Wo64 @ Wv64).T.astype(np.float32))
    bvo = (np.asarray(bv, np.float64) @ Wo64.T + np.asarray(bo, np.float64)).astype(np.float32)
    Q2 = q @ WQK + vq

    is_low = kj < R_SPLIT
    deg_low = np.bincount(qi[is_low], minlength=N)
    deg_high = np.bincount(qi[~is_low], minlength=N)
    group_of = _balance_nodes(deg_low, deg_high)

    # slot within group
    order_nodes = np.argsort(group_of, kind="stable")
    slot_in_group = np.empty(N, np.int64)
    gstart = np.searchsorted(group_of[order_nodes], np.arange(TOTAL_GROUPS))
    gend = np.append(gstart[1:], N)
    for g in range(TOTAL_GROUPS):
        slot_in_group[order_nodes[gstart[g]:gend[g]]] = np.arange(gend[g] - gstart[g])

    lgroup_of = group_of % GROUPS_PER_CORE
    lslot_of = lgroup_of * GROUP_NODES + slot_in_group

    e_group = group_of[qi]
    e_half = (~is_low).astype(np.int64)
    edge_order = np.lexsort((kj, e_half, e_group))
    eg_sorted = e_group[edge_order]
    # boundaries per (group, half)
    eh_sorted = e_half[edge_order]
    key_sorted = eg_sorted * 2 + eh_sorted
    bnd = np.searchsorted(key_sorted, np.arange(TOTAL_GROUPS * 2 + 1))

    per_core = []
    nov_total = 0
    for c in range(NC):
        gidx_cols = np.zeros((16, IDX_COLS_PER_BULK * BULKS), np.int16)
        qcolT = np.full((128, CHUNKS_PER_CORE), -1.0, np.float32)
        ov_edges = []
        for lg in range(GROUPS_PER_CORE):
            g = c * GROUPS_PER_CORE + lg
            lo = edge_order[bnd[2 * g]:bnd[2 * g + 1]]
            hi = edge_order[bnd[2 * g + 1]:bnd[2 * g + 2]]
            ov = []
            if len(lo) > LOW_CAP:
                ov.append(lo[LOW_CAP:]); lo = lo[:LOW_CAP]
            if len(hi) > HIGH_CAP:
                ov.append(hi[HIGH_CAP:]); hi = hi[:HIGH_CAP]
            b, gb = divmod(lg, GROUPS_PER_BULK)
            lo_chunk0 = b * CHUNKS_PER_BULK + gb * LOW_CHUNKS
            hi_chunk0 = b * CHUNKS_PER_BULK + GROUPS_PER_BULK * LOW_CHUNKS + gb * HIGH_CHUNKS
            for (sel, cap, base_sub, chunk0, col0) in (
                (lo, LOW_CAP, 0, lo_chunk0, b * IDX_COLS_PER_BULK),
                (hi, HIGH_CAP, R_SPLIT, hi_chunk0, b * IDX_COLS_PER_BULK + LOW_IDX_COLS),
            ):
                idx = np.zeros(cap, np.int64)
                idx[:len(sel)] = kj[sel] - base_sub
                qc = np.full(cap, -1.0, np.float32)
                qc[:len(sel)] = slot_in_group[qi[sel]]
                nchunk = cap // 128
                qcolT[:, chunk0:chunk0 + nchunk] = qc.reshape(nchunk, 128).T
                pos0 = gb * cap
                pos = pos0 + np.arange(cap)
                gidx_cols[pos % 16, col0 + pos // 16] = idx.astype(np.int16)
            ov_edges.extend(ov)
        # host handles overflow edges exactly (vectorized)
        corr = np.zeros((LOCAL_NODES, 129), np.float64)
        if ov_edges:
            ovi = np.concatenate(ov_edges)
            nov_total += len(ovi)
            s = lslot_of[qi[ovi]]
            sc = np.einsum("ij,ij->i", Q2[qi[ovi]].astype(np.float64),
                           kv[kj[ovi]].astype(np.float64))
            ex = np.exp(sc)
            np.add.at(corr[:, :128], s,
                      ex[:, None] * kv[kj[ovi]].astype(np.float64))
            np.add.at(corr[:, 128], s, ex)
        sel_c = group_of // GROUPS_PER_CORE == c
        perm = np.zeros(LOCAL_NODES, np.int64)
        valid = np.zeros(LOCAL_NODES, bool)
        nodes_c = np.nonzero(sel_c)[0]
        perm_slots = lslot_of[nodes_c]
        perm[perm_slots] = nodes_c
        valid[perm_slots] = True
        q2tT = np.ascontiguousarray(Q2[perm].T.astype(BF16))   # [128, 6272]
        per_core.append(dict(
            gidx=np.tile(gidx_cols, (8, 1)).astype(np.int16),
            qcolT=qcolT.astype(BF16), corr=corr.astype(np.float32).astype(BF16),
            q2tT=q2tT,
            q_local=np.ascontiguousarray(q[perm]),
            perm=perm, valid=valid,
        ))
    consts = dict(WvoT=np.ascontiguousarray(WvoT.astype(BF16)), bvo=bvo,
                  kv_lo=np.ascontiguousarray(kv[:R_SPLIT].astype(BF16)),
                  kv_hi=np.ascontiguousarray(kv[R_SPLIT:].astype(BF16)))
    return per_core, consts, nov_total


# ---------------- bass program ----------------
def build_program(skip_gather=False):
    import concourse.bacc as bacc
    import concourse.tile as tile
    from concourse import mybir

    f32 = mybir.dt.float32
    bf16 = mybir.dt.bfloat16
    nc = bacc.Bacc("TRN2", target_bir_lowering=False, debug=False,
                   num_swdge_queues=N_GATHER_QUEUES)

    kvlo_d = nc.dram_tensor("kv_lo", [R_SPLIT, D], bf16, kind="ExternalInput")
    kvhi_d = nc.dram_tensor("kv_hi", [N - R_SPLIT, D], bf16, kind="ExternalInput")
    q2t_d = nc.dram_tensor("q2tT", [128, LOCAL_NODES], bf16, kind="ExternalInput")
    corr_d = nc.dram_tensor("corr", [LOCAL_NODES, 129], bf16, kind="ExternalInput")
    id64_d = nc.dram_tensor("ident64", [64, 64], bf16, kind="ExternalInput")
    gidx_d = nc.dram_tensor("gidx", [128, IDX_COLS_PER_BULK * BULKS], mybir.dt.int16, kind="ExternalInput")
    qcol_d = nc.dram_tensor("qcolT", [128, CHUNKS_PER_CORE], bf16, kind="ExternalInput")
    wvo_d = nc.dram_tensor("WvoT", [D, D], bf16, kind="ExternalInput")
    iota_d = nc.dram_tensor("iota64", [128, GROUP_NODES], bf16, kind="ExternalInput")
    id128_d = nc.dram_tensor("ident128", [128, 128], bf16, kind="ExternalInput")
    out_d = nc.dram_tensor("y_out", [LOCAL_NODES, D], f32, kind="ExternalOutput")

    AluOp = mybir.AluOpType
    Act = mybir.ActivationFunctionType

    # exp batching: chunks per group processed in score-batches of 4
    EXP_BATCH = 4

    with tile.TileContext(nc) as tc:
        with (
            tc.tile_pool(name="persist", bufs=1) as pp,
            tc.tile_pool(name="gbuf", bufs=2) as gp,
            tc.tile_pool(name="work", bufs=4) as wp,
            tc.tile_pool(name="corrbuf", bufs=2) as cp,
            tc.tile_pool(name="fin", bufs=3) as fp_,
            tc.tile_pool(name="nctx", bufs=2) as np_,
            tc.tile_pool(name="ps_kvT", bufs=2, space="PSUM") as ps_kvT,
            tc.tile_pool(name="ps_M", bufs=3, space="PSUM") as ps_M,
            tc.tile_pool(name="ps_acc", bufs=2, space="PSUM") as ps_acc,
            tc.tile_pool(name="ps_fin", bufs=1, space="PSUM") as ps_fin,
        ):
            # persistent tiles; gidx is DMA'd per bulk (first bulk's slice
            # lands fast so gathers can start ASAP)
            gidx = pp.tile([128, IDX_COLS_PER_BULK * BULKS], mybir.dt.int16)
            wvo = pp.tile([D, D], bf16)
            iota = pp.tile([128, GROUP_NODES], bf16)
            id128 = pp.tile([128, 128], bf16)
            id64 = pp.tile([64, 64], bf16)
            ones = pp.tile([128, 1], bf16)
            nc.vector.memset(ones[:], 1.0)
            qcol = pp.tile([128, CHUNKS_PER_CORE], bf16)
            q2t = pp.tile([128, LOCAL_NODES], bf16)      # Q2^T [d, n]
            outbuf = pp.tile([128, (GROUPS_PER_CORE // 2) * 128], f32)

            def emit_persistent_loads():
                # emitted AFTER bulk 0's gidx slice + gathers so the first
                # gathers aren't queued behind ~1.7MB of constant loads
                nc.sync.dma_start(out=id128[:], in_=id128_d[:])
                nc.sync.dma_start(out=iota[:], in_=iota_d[:])
                nc.sync.dma_start(out=qcol[:], in_=qcol_d[:])
                nc.sync.dma_start(out=wvo[:], in_=wvo_d[:])
                nc.sync.dma_start(out=id64[:], in_=id64_d[:])
                nc.sync.dma_start(out=q2t[:], in_=q2t_d[:])

            gq = [0]  # gather queue round-robin counter

            def emit_bulk_loads(b):
                c0, c1 = b * IDX_COLS_PER_BULK, (b + 1) * IDX_COLS_PER_BULK
                nc.sync.dma_start(out=gidx[:, c0:c1], in_=gidx_d[:, c0:c1])
                gbuf = gp.tile([128, CHUNKS_PER_BULK * 128], bf16, tag="gbuf")
                glo = gbuf[:, :GROUPS_PER_BULK * LOW_CHUNKS * 128]
                ghi = gbuf[:, GROUPS_PER_BULK * LOW_CHUNKS * 128:]
                nlow = GROUPS_PER_BULK * LOW_CAP
                nhigh = GROUPS_PER_BULK * HIGH_CAP
                if skip_gather:
                    nc.vector.memset(gbuf[:], 1.0)
                else:
                    # HW limit: <=1024 idxs per dma_gather instruction.
                    # Interleave low/high gathers by group coverage so early
                    # groups become computable as soon as possible.
                    plan = []
                    for dst, src_d, total, col0, cap in (
                        (glo, kvlo_d, nlow, b * IDX_COLS_PER_BULK, LOW_CAP),
                        (ghi, kvhi_d, nhigh,
                         b * IDX_COLS_PER_BULK + LOW_IDX_COLS, HIGH_CAP),
                    ):
                        pos = 0
                        while pos < total:
                            n = min(1024, total - pos)
                            plan.append(((pos + n) / cap, dst, src_d,
                                         col0, pos, n))
                            pos += n
                    plan.sort(key=lambda t: t[0])
                    for _, dst, src_d, col0, pos, n in plan:
                        nc.gpsimd.dma_gather(
                            out_ap=dst[:, pos:pos + n]
                                .rearrange("p (c e) -> p c e", e=128),
                            in_ap=src_d[:],
                            idxs_ap=gidx[:, col0 + pos // 16:
                                         col0 + (pos + n) // 16],
                            num_idxs=n, num_idxs_reg=n, elem_size=D,
                            queue_num=gq[0] % N_GATHER_QUEUES)
                        gq[0] += 1
                corrbuf = cp.tile([64, GROUPS_PER_BULK * 129], bf16, tag="corr")
                nc.sync.dma_start(
                    out=corrbuf[:].rearrange("p (g c) -> p g c", g=GROUPS_PER_BULK),
                    in_=corr_d[b * GROUPS_PER_BULK * 64:(b + 1) * GROUPS_PER_BULK * 64, :]
                        .rearrange("(g p) c -> p g c", g=GROUPS_PER_BULK))
                return gbuf, corrbuf

            def chunk_ids_of(gb):
                return ([gb * LOW_CHUNKS + k for k in range(LOW_CHUNKS)] +
                        [GROUPS_PER_BULK * LOW_CHUNKS + gb * HIGH_CHUNKS + k
                         for k in range(HIGH_CHUNKS)])

            # software pipeline: batch k's transposes/scores overlap batch
            # k-1's exp/mask/aggregate work.
            batches = [(b, gb, b0)
                       for b in range(BULKS)
                       for gb in range(GROUPS_PER_BULK)
                       for b0 in range(0, CHUNKS_PER_GROUP, EXP_BATCH)]
            live = {}          # k -> dict(m_ps, gbuf, corrbuf, ...)
            pair_acc = {}      # pair -> psum acc tile [64, 258]
            pair_nctx = {}     # pair -> nctx2 sbuf tile
            bulk_tiles = {}    # b -> (gbuf, corrbuf)

            def stage_t(k):
                b, gb, b0 = batches[k]
                if b not in bulk_tiles:
                    bulk_tiles[b] = emit_bulk_loads(b)
                gbuf, corrbuf = bulk_tiles[b]
                ids = chunk_ids_of(gb)[b0:b0 + EXP_BATCH]
                kvT_ps = ps_kvT.tile([128, 128 * EXP_BATCH], bf16, tag="kvT")
                for ci, kc in enumerate(ids):
                    nc.tensor.transpose(out=kvT_ps[:, ci * 128:(ci + 1) * 128],
                                        in_=gbuf[:, kc * 128:(kc + 1) * 128],
                                        identity=id128[:])
                live[k] = dict(kvT_ps=kvT_ps, gbuf=gbuf, corrbuf=corrbuf,
                               ids=ids)

            def stage_s(k):
                b, gb, b0 = batches[k]
                st = live[k]
                ids = st["ids"]
                nb = len(ids)
                lg = b * GROUPS_PER_BULK + gb
                kvT3 = wp.tile([128, 128 * nb], bf16, tag="kvT3")
                if k % 2 == 0:
                    nc.vector.tensor_copy(out=kvT3[:],
                                          in_=st["kvT_ps"][:, :128 * nb])
                else:
                    nc.scalar.copy(out=kvT3[:], in_=st["kvT_ps"][:, :128 * nb])
                m_ps = ps_M.tile([128, 64 * nb], f32, tag="mps")
                for ci in range(nb):
                    nc.tensor.matmul(out=m_ps[:, ci * 64:(ci + 1) * 64],
                                     lhsT=kvT3[:, ci * 128:(ci + 1) * 128],
                                     rhs=q2t[:, lg * 64:(lg + 1) * 64],
                                     start=True, stop=True)
                st["m_ps"] = m_ps

            def stage_agg(k):
                b, gb, b0 = batches[k]
                st = live.pop(k)
                gbuf, corrbuf, ids = st["gbuf"], st["corrbuf"], st["ids"]
                lg = b * GROUPS_PER_BULK + gb
                pair, half = lg // 2, lg % 2
                nb = len(ids)
                expm = wp.tile([128, 64 * nb], bf16, tag="expm")
                nc.scalar.activation(out=expm[:], in_=st["m_ps"][:], func=Act.Exp)
                if b0 == 0:
                    acc = ps_acc.tile([64, 129], f32, tag="acc")
                    pair_acc[lg] = acc
                    # single start=True opens has_written for the whole bank;
                    # also folds in the host overflow correction.
                    nc.tensor.matmul(
                        out=acc[:], lhsT=id64[:],
                        rhs=corrbuf[:, gb * 129:(gb + 1) * 129],
                        start=True, stop=True)
                acc = pair_acc[lg]
                for ci, kc in enumerate(ids):
                    cglob = b * CHUNKS_PER_BULK + kc
                    kvchunk = gbuf[:, kc * 128:(kc + 1) * 128]
                    wmat = wp.tile([128, 64], bf16, tag="wmat")
                    nc.vector.scalar_tensor_tensor(
                        out=wmat[:], in0=iota[:],
                        scalar=qcol[:, cglob:cglob + 1],
                        in1=expm[:, ci * 64:(ci + 1) * 64],
                        op0=AluOp.is_equal, op1=AluOp.mult)
                    last = b0 + ci == CHUNKS_PER_GROUP - 1
                    nc.tensor.matmul(out=acc[:, 0:128], lhsT=wmat[:],
                                     rhs=kvchunk, start=False, stop=last,
                                     skip_group_check=True)
                    nc.tensor.matmul(out=acc[:, 128:129], lhsT=wmat[:],
                                     rhs=ones[:], start=False, stop=last,
                                     skip_group_check=True)
                if b0 + EXP_BATCH >= CHUNKS_PER_GROUP:
                    # group finalize: normalize (corr already in acc)
                    recip = fp_.tile([64, 1], f32, tag="recip")
                    nc.vector.reciprocal(out=recip[:], in_=acc[:, 128:129])
                    if half == 0:
                        nctx2 = np_.tile([128, 128], bf16, tag="nctx2")
                        pair_nctx[pair] = nctx2
                    nctx2 = pair_nctx[pair]
                    # normalize on ACT: out = Copy(acc * recip)
                    nc.scalar.activation(
                        out=nctx2[half * 64:(half + 1) * 64, :],
                        in_=acc[:, 0:128], func=Act.Copy, scale=recip[:])
                    if half == 1:
                        # nctxT (bf16) and y (f32) share one PSUM bank; the
                        # transpose -> copy -> matmul -> copy chain is strictly
                        # serial anyway, so the shared bank costs nothing.
                        fin_ps = ps_fin.tile([128, 768], mybir.dt.uint8,
                                             tag="fin")
                        nctxT_ps = fin_ps[:, 0:256].bitcast(bf16)
                        y_ps = fin_ps[:, 256:768].bitcast(f32)
                        nc.tensor.transpose(out=nctxT_ps[:], in_=nctx2[:],
                                            identity=id128[:])
                        nctxT = wp.tile([128, 128], bf16, tag="nctxT_sb")
                        nc.vector.tensor_copy(out=nctxT[:], in_=nctxT_ps[:])
                        nc.tensor.matmul(out=y_ps[:], lhsT=nctxT[:], rhs=wvo[:],
                                         start=True, stop=True)
                        nc.scalar.copy(
                            out=outbuf[:, pair * 128:(pair + 1) * 128],
                            in_=y_ps[:])
                        # stream this pair's rows out now instead of one big
                        # tail DMA at the end
                        nc.sync.dma_start(
                            out=out_d[pair * 128:(pair + 1) * 128, :],
                            in_=outbuf[:, pair * 128:(pair + 1) * 128])
                        del pair_nctx[pair]
                    del pair_acc[lg]

            # bulk 0's index slice + gathers go first, then constants
            bulk_tiles[0] = emit_bulk_loads(0)
            emit_persistent_loads()

            # per iteration: transposes(k) first (PE work that's ready),
            # then aggs(k-2) (deps long since satisfied), then copy+scores(k).
            LOOKAHEAD = 2
            for k in range(len(batches)):
                stage_t(k)
                if k >= LOOKAHEAD:
                    stage_agg(k - LOOKAHEAD)
                stage_s(k)
            for k in range(len(batches) - LOOKAHEAD, len(batches)):
                stage_agg(k)
    nc.compile()
    return nc


_PROGRAM_CACHE = {}


def _make_in_maps(per_core, consts):
    iota64 = np.tile(np.arange(GROUP_NODES, dtype=np.float32), (128, 1)).astype(BF16)
    id128 = np.eye(128, dtype=np.float32).astype(BF16)
    in_maps = []
    for c in range(NC):
        pc = per_core[c]
        in_maps.append({
            "kv_lo": consts["kv_lo"],
            "kv_hi": consts["kv_hi"],
            "q2tT": pc["q2tT"],
            "corr": pc["corr"],
            "gidx": pc["gidx"],
            "qcolT": pc["qcolT"],
            "WvoT": consts["WvoT"],
            "iota64": iota64,
            "ident128": id128,
            "ident64": np.eye(64, dtype=np.float32).astype(BF16),
        })
    return in_maps


def _collect(res, per_core, consts):
    out_full = np.zeros((N, D), np.float32)
    for c in range(NC):
        pc = per_core[c]
        y = np.asarray(res.results[c]["y_out"])
        v = pc["valid"]
        out_full[pc["perm"][v]] = y[v] + pc["q_local"][v]
    out_full += consts["bvo"]
    return out_full


def kernel(**inputs) -> np.ndarray:
    per_core, consts, _nov = host_prepare(**inputs)
    if "nc" not in _PROGRAM_CACHE:
        _PROGRAM_CACHE["nc"] = build_program()
    nc = _PROGRAM_CACHE["nc"]
    from concourse import bass_utils
    res = bass_utils.run_bass_kernel_spmd(nc, _make_in_maps(per_core, consts),
                                          core_ids=list(range(NC)))
    return _collect(res, per_core, consts)


def kernel_profiled(**inputs):
    """Same as kernel() but runs with trace=True and prints HW exec time."""
    per_core, consts, _nov = host_prepare(**inputs)
    if "nc" not in _PROGRAM_CACHE:
        _PROGRAM_CACHE["nc"] = build_program()
    nc = _PROGRAM_CACHE["nc"]
    from concourse import bass_utils
    res = bass_utils.run_bass_kernel_spmd(nc, _make_in_maps(per_core, consts),
                                          core_ids=list(range(NC)), trace=True)
    if res.exec_time_ns is not None:
        print(f"HW exec time: {res.exec_time_ns} ns")
    else:
        print("HW exec time: unavailable (no NTFF hook)")
    return _collect(res, per_core, consts)
